# revision 19
# baseline (speedup 1.0000x reference)
"""Trainium2 Bass kernel: dense multi-head dot-product attention.

Problem: x [4, 2048, 1024], W_Q/W_K/W_V [16, 1024, 64] ->
         out [4, 2048, 1024] (heads concatenated on the feature dim).

Sharding: 8 cores = 4 batches x 2 head-groups (8 heads each).
Per core, everything is computed in "transposed" layouts so that no
on-chip transpose of the big attention matrix is ever needed:
  - host passes x^T [1024, 2048] (n on partitions) per batch
  - projections (W stationary): Q^T/K^T/V^T [heads*64, 2048]
  - scores S^T[k, m] = sum_d K^T[d,k] Q^T[d,m]  (k on partitions)
  - P^T = exp(S^T/8)  (elementwise, ScalarE, PSUM->SBUF)
  - O^T[d, m] = sum_k Vaug[k, d] P^T[k, m] with Vaug = [V | ones],
    so row 64 of the accumulator is the softmax denominator.
  - normalize: recip(row64) broadcast over partitions (GpSimd), DVE mul
  - output O^T [512, 2048] per core; host transposes when gathering.
Softmax skips the max-subtraction: |S/8| < ~12 here, exp is safe in fp32
and softmax is shift-invariant, so the result is mathematically identical.

Matmul operands are float32r (fp32 bits, PE rounds to a reduced-precision
mode internally, ~1e-4 rel err, ~1 cyc/row at free-dim 512) by default.
PSUM accumulation stays fp32. KERNEL_MM_DTYPE=bf16|f32 to override.

The attention loop is split into m-halves of 1024 so that PSUM fits:
  shared tag (S^T chunks / proj accum / V-transposes)
                     3 bufs x [128,1024] fp32 = 6 banks
  ot (O^T accum)     1 buf  x [65, 1024] fp32 = 2 banks
Three rotating bufs let the PE run two score chunks ahead of ScalarE's
exp and let next-pair projections interleave with current attention.
"""

import os
from contextlib import ExitStack

import numpy as np

import concourse.bass as bass  # noqa: F401  (bass types via bacc)
import concourse.tile as tile
from concourse import bacc, mybir
from concourse import bass_utils
from concourse.masks import make_identity

F32 = mybir.dt.float32
F32R = mybir.dt.float32r
BF16 = mybir.dt.bfloat16

B, M, N, H, D = 4, 2048, 1024, 16, 64
HPC = 8          # heads per core
NCORES = 8
NCH = 8          # d_model / 128 chunks
KC = 16          # key chunks of 128
SCALE = 0.125    # 1/sqrt(64)
MH = 1024        # m-half width

_MM_DT = os.environ.get("KERNEL_MM_DTYPE", "f32r")
TMM = {"bf16": BF16, "f32r": F32R, "f32": F32}[_MM_DT]


def build_nc():
    nc = bacc.Bacc(
        "TRN2", target_bir_lowering=False, debug=False, enable_asserts=False
    )
    xt_d = nc.dram_tensor("xt", [N, M], F32, kind="ExternalInput")
    wq_d = nc.dram_tensor("wq", [4, N, 128], F32, kind="ExternalInput")
    wk_d = nc.dram_tensor("wk", [4, N, 128], F32, kind="ExternalInput")
    wv_d = nc.dram_tensor("wv", [4, N, 128], F32, kind="ExternalInput")
    o_d = nc.dram_tensor("ot", [HPC * D, M], F32, kind="ExternalOutput")

    with tile.TileContext(nc) as tc, ExitStack() as ctx:
        const_pool = ctx.enter_context(tc.tile_pool(name="constp", bufs=1))
        xt_pool = ctx.enter_context(tc.tile_pool(name="xtp", bufs=NCH))
        w_pool = ctx.enter_context(tc.tile_pool(name="wp", bufs=3))
        qkv_pool = ctx.enter_context(tc.tile_pool(name="qkvp", bufs=2))
        vaug_pool = ctx.enter_context(tc.tile_pool(name="vaugp", bufs=2))
        pt_pool = ctx.enter_context(tc.tile_pool(name="ptp", bufs=4))
        out_pool = ctx.enter_context(tc.tile_pool(name="outp", bufs=4))
        small_pool = ctx.enter_context(tc.tile_pool(name="smallp", bufs=3))
        # PSUM: shared tag (st chunks / proj accum / transposes) 3x2 banks
        # + ot 1x2 banks = 8 banks. Three bufs let the PE run two score
        # chunks ahead of ScalarE's exp, hiding the exp latency.
        st_pool = ctx.enter_context(tc.tile_pool(name="stp", bufs=3, space="PSUM"))
        ot_pool = ctx.enter_context(tc.tile_pool(name="otp", bufs=1, space="PSUM"))

        # memset/affine_select cannot target f32r, and transpose operands
        # must share a dtype: keep the V^T -> V transpose in plain f32
        # (the copy into vaug rounds to TMM).
        TID = BF16 if TMM == BF16 else F32
        ident = const_pool.tile([128, 128], TID, name="ident")
        make_identity(nc, ident[:])
        ones16 = const_pool.tile([128, 16, 1], F32, name="ones16")
        nc.gpsimd.memset(ones16[:], 1.0)

        # ---- resident x^T tiles; SWDGE DMA casts fp32 -> TMM on load
        # load in m-quarters, first quarter for all chunks first, so the
        # first projection matmuls start after 2MB instead of 8MB.
        xts = []
        for c in range(NCH):
            xtile = xt_pool.tile([128, M], TMM, name=f"xt{c}", tag="xtile")
            nc.gpsimd.dma_start(
                xtile[:, 0:512], xt_d.ap()[c * 128:(c + 1) * 128, 0:512]
            )
            xts.append(xtile)
        for q in range(1, 4):
            for c in range(NCH):
                nc.gpsimd.dma_start(
                    xts[c][:, q * 512:(q + 1) * 512],
                    xt_d.ap()[c * 128:(c + 1) * 128, q * 512:(q + 1) * 512],
                )

        for p in range(4):  # head pairs
            # ---- projections: dst[h%2*64+d, m] for the two heads of pair p
            qkv = {}
            for nm, wd in (("q", wq_d), ("k", wk_d), ("v", wv_d)):
                wt = w_pool.tile([128, NCH, 128], TMM, name=f"wt_{nm}", tag="wt")
                nc.gpsimd.dma_start(
                    wt[:], wd.ap()[p].rearrange("(c p) d -> p c d", p=128)
                )
                ddt = TMM if nm in ("q", "k") else TID
                dst = qkv_pool.tile([128, M], ddt, name=f"{nm}t", tag=f"{nm}t")
                for mh in range(2):
                    ps = st_pool.tile([128, MH], F32, name="ps_prj", tag="st")
                    for c in range(NCH):
                        for mc in range(2):
                            nc.tensor.matmul(
                                ps[:, mc * 512:(mc + 1) * 512],
                                lhsT=wt[:, c, :],
                                rhs=xts[c][
                                    :,
                                    mh * MH + mc * 512: mh * MH + (mc + 1) * 512,
                                ],
                                start=(c == 0),
                                stop=(c == NCH - 1),
                                skip_group_check=True,
                            )
                    nc.vector.tensor_copy(dst[:, mh * MH:(mh + 1) * MH], ps[:])
                qkv[nm] = dst
            qt, kt, vt = qkv["q"], qkv["k"], qkv["v"]

            # ---- Vaug[k, kc, hp*65 + d]; col hp*65+64 = 1.0 (ones col)
            vaug = vaug_pool.tile([128, KC, 130], TMM, name="vaug", tag="vaug")
            for hp in range(2):
                nc.vector.tensor_copy(
                    vaug[:, :, hp * 65 + 64:hp * 65 + 65], ones16[:]
                )
            for kc in range(KC):
                trp = st_pool.tile([128, 128], TID, name="trp", tag="st")
                nc.tensor.transpose(
                    trp[:], vt[:, kc * 128:(kc + 1) * 128], ident[:]
                )
                nc.vector.tensor_copy(
                    vaug[:, kc, :].rearrange("p (h x) -> p h x", h=2)[:, :, 0:64],
                    trp.rearrange("p (h d) -> p h d", h=2),
                )

            # ---- attention per head, split in m-halves of 1024
            for hp in range(2):
                h = 2 * p + hp
                hsl = slice(64 * hp, 64 * (hp + 1))
                for mh in range(2):
                    mbase = mh * MH
                    ot = ot_pool.tile([65, MH], F32, name="ot", tag="ot")
                    for kc in range(KC):
                        st = st_pool.tile([128, MH], F32, name="st", tag="st")
                        for mc in range(2):
                            nc.tensor.matmul(
                                st[:, mc * 512:(mc + 1) * 512],
                                lhsT=kt[hsl, kc * 128:(kc + 1) * 128],
                                rhs=qt[
                                    hsl,
                                    mbase + mc * 512: mbase + (mc + 1) * 512,
                                ],
                                start=True,
                                stop=True,
                            )
                        pt = pt_pool.tile([128, MH], TMM, name="pt", tag="pt")
                        nc.scalar.activation(
                            pt[:], st[:],
                            mybir.ActivationFunctionType.Exp, scale=SCALE,
                        )
                        for mc in range(2):
                            nc.tensor.matmul(
                                ot[:, mc * 512:(mc + 1) * 512],
                                lhsT=vaug[:, kc, hp * 65:hp * 65 + 65],
                                rhs=pt[:, mc * 512:(mc + 1) * 512],
                                start=(kc == 0),
                                stop=(kc == KC - 1),
                                skip_group_check=True,
                            )
                    # ---- normalize rows 0..63 by row 64; free ot ASAP
                    sumsb = small_pool.tile([1, MH], F32, name="sumsb", tag="sm")
                    nc.vector.tensor_copy(sumsb[:], ot[64:65, :])
                    ostage = out_pool.tile([64, MH], F32, name="ostage", tag="o64")
                    nc.vector.tensor_copy(ostage[:], ot[0:64, :])
                    recipb = small_pool.tile([1, MH], F32, name="recipb", tag="sm")
                    scratch = small_pool.tile([1, MH], F32, name="scr", tag="sm")
                    nc.vector.reciprocal_approx_accurate(
                        recipb[:], sumsb[:], scratch[:]
                    )
                    rbc = out_pool.tile([64, MH], F32, name="rbc", tag="o64")
                    nc.gpsimd.partition_broadcast(rbc[:], recipb[:])
                    stage = out_pool.tile([64, MH], F32, name="stage", tag="o64")
                    nc.vector.tensor_mul(stage[:], ostage[:], rbc[:])
                    nc.sync.dma_start(
                        o_d.ap()[h * 64:(h + 1) * 64, mbase:mbase + MH], stage[:]
                    )
    nc.compile()
    return nc


_NC_CACHE = None


def _get_nc():
    global _NC_CACHE
    if _NC_CACHE is None:
        _NC_CACHE = build_nc()
    return _NC_CACHE


def make_in_maps(x, W_Q, W_K, W_V):
    x = np.asarray(x, dtype=np.float32)
    W_Q = np.asarray(W_Q, dtype=np.float32)
    W_K = np.asarray(W_K, dtype=np.float32)
    W_V = np.asarray(W_V, dtype=np.float32)

    def prep_w(W, g):
        blk = W[8 * g:8 * g + 8]  # [8, 1024, 64]
        # pair-major [4, 1024, 128]: col = (head%2)*64 + d
        return np.ascontiguousarray(
            blk.reshape(4, 2, N, D).transpose(0, 2, 1, 3).reshape(4, N, 2 * D)
        )

    in_maps = []
    for c in range(NCORES):
        b, g = divmod(c, 2)
        in_maps.append(
            {
                "xt": np.ascontiguousarray(x[b].T),
                "wq": prep_w(W_Q, g),
                "wk": prep_w(W_K, g),
                "wv": prep_w(W_V, g),
            }
        )
    return in_maps


def gather_out(results):
    out = np.empty((B, M, N), dtype=np.float32)
    for c in range(NCORES):
        b, g = divmod(c, 2)
        out[b, :, 512 * g:512 * (g + 1)] = results[c]["ot"].T
    return out


def run(x, W_Q, W_K, W_V, **spmd_kwargs):
    nc = _get_nc()
    in_maps = make_in_maps(x, W_Q, W_K, W_V)
    res = bass_utils.run_bass_kernel_spmd(
        nc, in_maps, core_ids=list(range(NCORES)), **spmd_kwargs
    )
    return gather_out(res.results), res


def kernel(x, W_Q, W_K, W_V):
    out, _ = run(x, W_Q, W_K, W_V)
    return out
